# revision 1
# baseline (speedup 1.0000x reference)
"""Trainium2 Bass kernel for the 6-layer post-LN transformer encoder.

Data-parallel over batch: 8 NeuronCores x 2 batches each, weights replicated,
no collectives.  Activations are kept feature-major ``hT[d, token]`` so every
linear layer runs with weight tiles stationary; LayerNorm statistics are
computed with ones-vector matmuls on the PE.  Matmuls run in float32r
(full-rate fp32, ~1e-4 rounding) except Q/K score matmuls which run in bf16.

The reference faithfully replicates torch's buggy ``.view(B*H, -1, Dh)`` head
split, so attention operates on 128 "pseudo-groups" (batch x 64-token block)
of 512 pseudo-positions j = (s%64)*8 + h, and group ``i`` uses the padding
mask of batch ``i % B``.  Scores are computed transposed (pseudo-keys on
partitions, block order jk' = hk*64 + smk) so the padding mask folds into the
Exp bias and the softmax denominator comes from a ones-column on V.
"""

import os
import sys

import numpy as np

for _p in ("/opt/trn_rl_repo", "/root/.axon_site/_ro/trn_rl_repo"):
    if os.path.isdir(_p) and _p not in sys.path:
        sys.path.append(_p)

import concourse.bass as bass
import concourse.mybir as mybir
from concourse import bacc
from concourse.tile import TileContext
from concourse.bass_utils import run_bass_kernel_spmd
from concourse.masks import make_identity

B, S, D, H, Dh, F, L, V = 16, 512, 512, 8, 64, 2048, 6, 32000
NCORES = 8
BPC = B // NCORES          # batches per core
NT = BPC * S               # tokens per core
DT = D // 128              # d-dim partition tiles
FT = F // 128              # ffn-dim partition tiles
NG = BPC * 8               # pseudo attention groups per core
f32 = mybir.dt.float32
f32r = mybir.dt.float32r
bf16 = mybir.dt.bfloat16
AF = mybir.ActivationFunctionType

_CACHE = {}
_UID = [0]


def _nm(p):
    _UID[0] += 1
    return f"{p}{_UID[0]}"


def _build():
    nc = bacc.Bacc(None, target_bir_lowering=False)

    IDX = nc.dram_tensor("IDX", [128, NT // 16], mybir.dt.int16, kind="ExternalInput")
    PEM = nc.dram_tensor("PEM", [DT, 128, NT], f32, kind="ExternalInput")
    KB = nc.dram_tensor("KB", [128, 16 * 4], f32, kind="ExternalInput")
    EMB = nc.dram_tensor("EMB", [V, D], f32, kind="ExternalInput")
    WQ = nc.dram_tensor("WQ", [L, DT, DT, 128, 128], f32r, kind="ExternalInput")
    WK = nc.dram_tensor("WK", [L, DT, DT, 128, 128], f32r, kind="ExternalInput")
    WV = nc.dram_tensor("WV", [L, DT, 128, D], f32r, kind="ExternalInput")
    WO = nc.dram_tensor("WO", [L, DT, DT, 128, 128], f32r, kind="ExternalInput")
    W1 = nc.dram_tensor("W1", [L, DT, FT, 128, 128], f32r, kind="ExternalInput")
    W2 = nc.dram_tensor("W2", [L, FT, DT, 128, 128], f32r, kind="ExternalInput")
    # biases / gains, host-tiled to [L, 128, ntiles]
    BO = nc.dram_tensor("BO", [L, 128, DT], f32, kind="ExternalInput")
    B1 = nc.dram_tensor("B1", [L, 128, FT], f32, kind="ExternalInput")
    B2 = nc.dram_tensor("B2", [L, 128, DT], f32, kind="ExternalInput")
    G1 = nc.dram_tensor("G1", [L, 128, DT], f32, kind="ExternalInput")
    E1 = nc.dram_tensor("E1", [L, 128, DT], f32, kind="ExternalInput")
    G2 = nc.dram_tensor("G2", [L, 128, DT], f32, kind="ExternalInput")
    E2 = nc.dram_tensor("E2", [L, 128, DT], f32, kind="ExternalInput")
    OUT = nc.dram_tensor("OUT", [NT, D], f32, kind="ExternalOutput")

    from contextlib import ExitStack

    with TileContext(nc) as tc:
        with ExitStack() as st:
            act_pool = st.enter_context(tc.tile_pool(name="act", bufs=8))
            emb_pool = st.enter_context(tc.tile_pool(name="emb", bufs=1))
            tmp_pool = st.enter_context(tc.tile_pool(name="tmp", bufs=3))
            ctx_pool = st.enter_context(tc.tile_pool(name="ctx", bufs=4))
            qk_pool = st.enter_context(tc.tile_pool(name="qk", bufs=1))
            v_pool = st.enter_context(tc.tile_pool(name="vst", bufs=16))
            e_pool = st.enter_context(tc.tile_pool(name="ebuf", bufs=3))
            eo_pool = st.enter_context(tc.tile_pool(name="eodd", bufs=3))
            f_pool = st.enter_context(tc.tile_pool(name="fbuf", bufs=4))
            uw_pool = st.enter_context(tc.tile_pool(name="uw", bufs=1))
            wt_pool = st.enter_context(tc.tile_pool(name="wt", bufs=12))
            wv_pool = st.enter_context(tc.tile_pool(name="wv4", bufs=4))
            sm_pool = st.enter_context(tc.tile_pool(name="small", bufs=2))
            cst_pool = st.enter_context(tc.tile_pool(name="cst", bufs=1))
            pbig = st.enter_context(tc.tile_pool(name="pbig", bufs=2, space="PSUM"))
            pacc = st.enter_context(tc.tile_pool(name="pacc", bufs=4, space="PSUM"))
            # ---- constants ----
            ident = cst_pool.tile([128, 128], f32, tag="identf")
            make_identity(nc, ident[:, :])
            identr = cst_pool.tile([128, 128], f32r, tag="identr")
            nc.vector.tensor_copy(identr[:, :], ident[:, :])
            ones_f = cst_pool.tile([128, 9], f32, tag="onesf")
            nc.gpsimd.memset(ones_f[:, :], 1.0)
            ones_r = cst_pool.tile([128, 1], f32r, tag="ones")
            nc.vector.tensor_copy(ones_r[:, :], ones_f[:, 0:1])
            kb_sb = cst_pool.tile([128, 64], f32, tag="kb")
            nc.sync.dma_start(kb_sb[:, :], KB[:, :])
            idx_sb = cst_pool.tile([128, NT // 16], mybir.dt.int16, tag="idx")
            nc.sync.dma_start(idx_sb[:, :], IDX[:, :])
            eps_sb = cst_pool.tile([1, 1], f32, tag="eps")
            nc.gpsimd.memset(eps_sb[:, :], 1e-5)

            # ---- embedding: gather + transpose + pe add ----
            hT = [act_pool.tile([128, NT], f32r, tag="act", name=_nm("hT")) for _ in range(DT)]
            pem_sb = []
            for dt in range(DT):
                p = act_pool.tile([128, NT], f32, tag="act")
                nc.sync.dma_start(p[:, :], PEM[dt, :, :])
                pem_sb.append(p)
            for half in range(2):
                g_sb = emb_pool.tile([128, NT // 256, D], f32, tag="gsb")
                nc.gpsimd.dma_gather(
                    g_sb[:, :, :], EMB[:, :],
                    idx_sb[:, half * (NT // 32):(half + 1) * (NT // 32)],
                    NT // 2, NT // 2, D,
                )
                for ch in range(NT // 256):
                    c = half * (NT // 256) + ch
                    for dt in range(DT):
                        pt = pacc.tile([128, 128], f32, tag="pacc")
                        nc.tensor.transpose(
                            pt[:, :], g_sb[:, ch, dt * 128:(dt + 1) * 128], ident[:, :]
                        )
                        nc.vector.tensor_add(
                            hT[dt][:, c * 128:(c + 1) * 128],
                            pt[:, :],
                            pem_sb[dt][:, c * 128:(c + 1) * 128],
                        )

            # ---- layers ----
            for l in range(L):
                # --- load per-layer bias tiles ---
                bo_sb = sm_pool.tile([128, DT], f32, tag="bo")
                nc.sync.dma_start(bo_sb[:, :], BO[l, :, :])
                b1_sb = sm_pool.tile([128, FT], f32, tag="b1")
                nc.sync.dma_start(b1_sb[:, :], B1[l, :, :])
                b2_sb = sm_pool.tile([128, DT], f32, tag="b2")
                nc.sync.dma_start(b2_sb[:, :], B2[l, :, :])
                g1_sb = sm_pool.tile([128, DT], f32, tag="g1")
                nc.sync.dma_start(g1_sb[:, :], G1[l, :, :])
                e1_sb = sm_pool.tile([128, DT], f32, tag="e1")
                nc.sync.dma_start(e1_sb[:, :], E1[l, :, :])
                g2_sb = sm_pool.tile([128, DT], f32, tag="g2")
                nc.sync.dma_start(g2_sb[:, :], G2[l, :, :])
                e2_sb = sm_pool.tile([128, DT], f32, tag="e2")
                nc.sync.dma_start(e2_sb[:, :], E2[l, :, :])

                # --- Q / K projections into head-stacked bf16 [64, H*NT] ---
                q_stack = qk_pool.tile([64, H * NT], bf16, tag="qs")
                k_stack = qk_pool.tile([64, H * NT], bf16, tag="ks")
                for W_dram, stack in ((WQ, q_stack), (WK, k_stack)):
                    for mt in range(DT):
                        ps = pbig.tile([128, NT], f32, tag="pbig")
                        for kt in range(DT):
                            w_t = wt_pool.tile([128, 128], f32r, tag="wt")
                            nc.sync.dma_start(w_t[:, :], W_dram[l, kt, mt, :, :])
                            for nch in range(NT // 512):
                                nc.tensor.matmul(
                                    ps[:, nch * 512:(nch + 1) * 512],
                                    w_t[:, :],
                                    hT[kt][:, nch * 512:(nch + 1) * 512],
                                    start=(kt == 0),
                                    stop=(kt == DT - 1),
                                )
                        nc.scalar.copy(
                            stack[:, (2 * mt) * NT:(2 * mt + 1) * NT], ps[0:64, :]
                        )
                        nc.scalar.copy(
                            stack[:, (2 * mt + 1) * NT:(2 * mt + 2) * NT],
                            ps[64:128, :],
                        )

                # --- V projection into token-major with ones column ---
                wv_sb = []
                for kt in range(DT):
                    w_t = wv_pool.tile([128, D], f32r, tag="wv")
                    nc.sync.dma_start(w_t[:, :], WV[l, kt, :, :])
                    wv_sb.append(w_t)
                v_st = []                      # one [64, H*65] tile per group
                for tt in range(NT // 128):
                    ps = pacc.tile([128, 512], f32, tag="pacc")
                    for kt in range(DT):
                        nc.tensor.matmul(
                            ps[:, :],
                            hT[kt][:, tt * 128:(tt + 1) * 128],
                            wv_sb[kt][:, :],
                            start=(kt == 0),
                            stop=(kt == DT - 1),
                        )
                    for half in range(2):
                        vt = v_pool.tile([64, H * 65], f32r, tag="vst")
                        nc.scalar.copy(
                            vt.rearrange("p (h e) -> p h e", h=H)[:, :, 0:64],
                            ps[half * 64:half * 64 + 64, :],
                        )
                        nc.vector.tensor_copy(
                            vt.rearrange("p (h e) -> p h e", h=H)[:, :, 64:65],
                            ones_f[0:64, 0:8].rearrange("p (h o) -> p h o", o=1),
                        )
                        v_st.append(vt)

                # --- attention per pseudo-group ---
                ctxT = [ctx_pool.tile([128, NT], f32r, tag="ctx", name=_nm("ctxT")) for _ in range(DT)]
                q_v = q_stack.rearrange("p (h t) -> p h t", h=H)
                for g in range(NG):
                    tb = g * 64                       # local token base
                    m = g % 16                        # mask row: 8*(b%2)+gb == g%16
                    sp = [pacc.tile([128, 512], f32, tag="pacc", name=_nm("sp")) for _ in range(4)]
                    rhs = q_v[:, :, tb:tb + 64]
                    for hk in range(H):
                        kt2 = hk // 2
                        nc.tensor.matmul(
                            sp[kt2][(hk % 2) * 64:(hk % 2) * 64 + 64, :],
                            k_stack[:, hk * NT + tb:hk * NT + tb + 64],
                            rhs,
                            start=True,
                            stop=True,
                        )
                    e4 = []
                    eo = []
                    for kt2 in range(4):
                        e_t = e_pool.tile([128, 512], f32r, tag="e4")
                        nc.scalar.activation(
                            e_t[:, :],
                            sp[kt2][:, :],
                            AF.Exp,
                            bias=kb_sb[:, m * 4 + kt2:m * 4 + kt2 + 1],
                            scale=1.0,
                        )
                        e4.append(e_t)
                        et = eo_pool.tile([64, 512], f32r, tag="eo")
                        nc.vector.tensor_copy(et[:, :], e_t[64:128, :])
                        eo.append(et)
                    cps = pbig.tile([128, 512], f32, tag="pbig", name=_nm("cps"))
                    for hk in range(H):
                        rhs_e = e4[hk // 2][0:64, :] if hk % 2 == 0 else eo[hk // 2][:, :]
                        nc.tensor.matmul(
                            cps[0:65, :],
                            v_st[g][:, hk * 65:hk * 65 + 65],
                            rhs_e,
                            start=(hk == 0),
                            stop=(hk == H - 1),
                        )
                    rec = sm_pool.tile([1, 512], f32, tag="rec", bufs=2)
                    nc.vector.reciprocal(rec[:, :], cps[64:65, :])
                    rb = uw_pool.tile([64, 512], f32, tag="rb")
                    nc.gpsimd.partition_broadcast(rb[:, :], rec[0:1, :])
                    for hq in range(H):
                        nc.vector.tensor_mul(
                            ctxT[hq // 2][(hq % 2) * 64:(hq % 2) * 64 + 64, tb:tb + 64],
                            cps[0:64, hq * 64:(hq + 1) * 64],
                            rb[:, hq * 64:(hq + 1) * 64],
                        )

                # --- Wo + bias + residual -> h_pre ---
                h_pre = [act_pool.tile([128, NT], f32r, tag="act", name=_nm("hpre")) for _ in range(DT)]
                for mt in range(DT):
                    ps = pbig.tile([128, NT], f32, tag="pbig")
                    for kt in range(DT):
                        w_t = wt_pool.tile([128, 128], f32r, tag="wt")
                        nc.sync.dma_start(w_t[:, :], WO[l, kt, mt, :, :])
                        for nch in range(NT // 512):
                            nc.tensor.matmul(
                                ps[:, nch * 512:(nch + 1) * 512],
                                w_t[:, :],
                                ctxT[kt][:, nch * 512:(nch + 1) * 512],
                                start=(kt == 0),
                                stop=(kt == DT - 1),
                            )
                    t_o = tmp_pool.tile([128, NT], f32r, tag="tmp")
                    nc.scalar.activation(
                        t_o[:, :], ps[:, :], AF.Identity,
                        bias=bo_sb[:, mt:mt + 1], scale=1.0,
                    )
                    nc.vector.tensor_add(h_pre[mt][:, :], t_o[:, :], hT[mt][:, :])

                h1 = _layernorm(nc, act_pool, tmp_pool, uw_pool, sm_pool, pacc,
                                ones_r, eps_sb, h_pre, g1_sb, e1_sb)

                # --- FFN ---
                h_pre2 = [act_pool.tile([128, NT], f32r, tag="act", name=_nm("hpre2")) for _ in range(DT)]
                for nch in range(NT // 512):
                    o2ps = [pacc.tile([128, 512], f32, tag="pacc", name=_nm("o2ps")) for _ in range(DT)]
                    for mtf in range(FT):
                        ps = pbig.tile([128, 512], f32, tag="pbig")
                        for kt in range(DT):
                            w_t = wt_pool.tile([128, 128], f32r, tag="wt")
                            nc.sync.dma_start(w_t[:, :], W1[l, kt, mtf, :, :])
                            nc.tensor.matmul(
                                ps[:, :],
                                w_t[:, :],
                                h1[kt][:, nch * 512:(nch + 1) * 512],
                                start=(kt == 0),
                                stop=(kt == DT - 1),
                            )
                        f_t = f_pool.tile([128, 512], f32r, tag="f")
                        nc.scalar.activation(
                            f_t[:, :], ps[:, :], AF.Relu,
                            bias=b1_sb[:, mtf:mtf + 1], scale=1.0,
                        )
                        for mtd in range(DT):
                            w_t2 = wt_pool.tile([128, 128], f32r, tag="wt")
                            nc.sync.dma_start(w_t2[:, :], W2[l, mtf, mtd, :, :])
                            nc.tensor.matmul(
                                o2ps[mtd][:, :],
                                w_t2[:, :],
                                f_t[:, :],
                                start=(mtf == 0),
                                stop=(mtf == FT - 1),
                            )
                    for mtd in range(DT):
                        t_o = tmp_pool.tile([128, 512], f32r, tag="tmp")
                        nc.scalar.activation(
                            t_o[:, :], o2ps[mtd][:, :], AF.Identity,
                            bias=b2_sb[:, mtd:mtd + 1], scale=1.0,
                        )
                        nc.vector.tensor_add(
                            h_pre2[mtd][:, nch * 512:(nch + 1) * 512],
                            t_o[:, :],
                            h1[mtd][:, nch * 512:(nch + 1) * 512],
                        )

                hT = _layernorm(nc, act_pool, tmp_pool, uw_pool, sm_pool, pacc,
                                ones_r, eps_sb, h_pre2, g2_sb, e2_sb)

            # ---- final transpose to token-major + store ----
            for c in range(NT // 128):
                o_sb = act_pool.tile([128, D], f32, tag="act")
                for dt in range(DT):
                    pt = pacc.tile([128, 128], f32r, tag="pacc")
                    nc.tensor.transpose(
                        pt[:, :], hT[dt][:, c * 128:(c + 1) * 128], identr[:, :]
                    )
                    nc.scalar.copy(o_sb[:, dt * 128:(dt + 1) * 128], pt[:, :])
                nc.sync.dma_start(OUT[c * 128:(c + 1) * 128, :], o_sb[:, :])

    nc.compile()
    return nc


def _layernorm(nc, act_pool, tmp_pool, uw_pool, sm_pool, pacc, ones_r, eps_sb, h_in, g_sb, b_sb):
    """Post-LN over the feature (partition) dim of feature-major tiles."""
    ms0 = sm_pool.tile([1, NT], f32r, tag="ms", bufs=1)
    ms1 = sm_pool.tile([1, NT], f32r, tag="ms1", bufs=1)
    for nch in range(NT // 512):
        sl = slice(nch * 512, (nch + 1) * 512)
        st0 = pacc.tile([1, 512], f32, tag="pacc", name=_nm("st0"))
        st1 = pacc.tile([1, 512], f32, tag="pacc", name=_nm("st1"))
        for kt in range(DT):
            nc.tensor.matmul(
                st0[0:1, :], ones_r[:, :], h_in[kt][:, sl],
                start=(kt == 0), stop=(kt == DT - 1),
            )
        for kt in range(DT):
            sq = tmp_pool.tile([128, 512], f32r, tag="tmp")
            nc.vector.tensor_mul(sq[:, :], h_in[kt][:, sl], h_in[kt][:, sl])
            nc.tensor.matmul(
                st1[0:1, :], ones_r[:, :], sq[:, :],
                start=(kt == 0), stop=(kt == DT - 1),
            )
        nc.scalar.mul(ms0[:, sl], st0[0:1, :], 1.0 / D)
        nc.scalar.mul(ms1[:, sl], st1[0:1, :], 1.0 / D)
    m2 = sm_pool.tile([1, NT], f32r, tag="lns", bufs=2, name=_nm("m2"))
    nc.vector.tensor_mul(m2[:, :], ms0[:, :], ms0[:, :])
    var = sm_pool.tile([1, NT], f32r, tag="lns", bufs=2, name=_nm("var"))
    nc.vector.tensor_sub(var[:, :], ms1[:, :], m2[:, :])
    sd = sm_pool.tile([1, NT], f32r, tag="lns", bufs=2, name=_nm("sd"))
    nc.scalar.activation(sd[:, :], var[:, :], AF.Sqrt, bias=eps_sb[0:1, 0:1], scale=1.0)
    inv = sm_pool.tile([1, NT], f32r, tag="inv", bufs=1)
    with nc.allow_low_precision(reason="f32r LN inv, ~1e-4 rounding is fine"):
        nc.vector.reciprocal(inv[:, :], sd[:, :])
    w = sm_pool.tile([1, NT], f32r, tag="w", bufs=1)
    nc.vector.tensor_mul(w[:, :], ms0[:, :], inv[:, :])
    U = uw_pool.tile([128, NT], f32r, tag="U")
    nc.gpsimd.partition_broadcast(U[:, :], inv[0:1, :])
    W = uw_pool.tile([128, NT], f32r, tag="W")
    nc.gpsimd.partition_broadcast(W[:, :], w[0:1, :])
    h_out = []
    for dt in range(DT):
        t1 = tmp_pool.tile([128, NT], f32r, tag="tmp")
        nc.vector.tensor_mul(t1[:, :], h_in[dt][:, :], U[:, :])
        nc.vector.tensor_sub(t1[:, :], t1[:, :], W[:, :])
        ho = act_pool.tile([128, NT], f32r, tag="act")
        nc.scalar.activation(
            ho[:, :], t1[:, :], AF.Identity,
            bias=b_sb[:, dt:dt + 1], scale=g_sb[:, dt:dt + 1],
        )
        h_out.append(ho)
    return h_out


def _host_prep(x, batch_length, embed, Wq, Wk, Wv, Wo, bo, ln1_g, ln1_b,
               W1, b1, W2, b2, ln2_g, ln2_b):
    x = np.asarray(x).astype(np.int64)
    batch_length = np.asarray(batch_length).astype(np.int64)
    f = lambda a: np.ascontiguousarray(np.asarray(a), dtype=np.float32)
    embed = f(embed)

    # sinusoidal PE, exact float32 replication of the reference formula
    pos = np.arange(S, dtype=np.float32)[:, None]
    i = np.arange(D, dtype=np.float32)[None, :]
    ang = (pos / np.power(np.float32(10000.0), (np.float32(2.0) * i / np.float32(D)),
                          dtype=np.float32)).astype(np.float32)
    pe = ang.copy()
    pe[:, 0::2] = np.sin(ang[:, 0::2])
    pe[:, 1::2] = np.cos(ang[:, 1::2])

    scale = np.float32(1.0) / np.sqrt(np.float32(Dh))

    # weights, pre-tiled [L, kt, 128, out]
    def tile_k(w):
        w = f(w)
        kt, mt = w.shape[1] // 128, w.shape[2] // 128
        return np.ascontiguousarray(
            w.reshape(L, kt, 128, mt, 128).transpose(0, 1, 3, 2, 4)
        )

    wq = tile_k(np.asarray(Wq) * scale)
    wk = tile_k(Wk)
    wv_full = f(Wv)
    wv = np.ascontiguousarray(wv_full.reshape(L, DT, 128, D))
    wo = tile_k(Wo)
    w1 = tile_k(W1)
    w2 = tile_k(W2)

    tile_b = lambda b, nt: np.ascontiguousarray(
        f(b).reshape(L, nt, 128).transpose(0, 2, 1)
    )
    bo_t = tile_b(bo, DT)
    b1_t = tile_b(b1, FT)
    b2_t = tile_b(b2, DT)
    g1_t = tile_b(ln1_g, DT)
    e1_t = tile_b(ln1_b, DT)
    g2_t = tile_b(ln2_g, DT)
    e2_t = tile_b(ln2_b, DT)

    # mask bias table in blocked pseudo-key order: kb[p, m*4+kt] for jk'=kt*128+p
    pad = (x == 0)
    kb = np.zeros((128, 64), np.float32)
    for m in range(16):
        for kt in range(4):
            jk_blk = kt * 128 + np.arange(128)          # jk' = hk*64+smk
            hk, smk = jk_blk // 64, jk_blk % 64
            jk = smk * 8 + hk                           # interleaved pseudo-pos
            kb[:, m * 4 + kt] = np.where(pad[m, jk], np.float32(-1e30), 0.0)

    in_maps = []
    for c in range(NCORES):
        bsel = slice(2 * c, 2 * c + 2)
        xs = x[bsel]                                    # [2, S]
        lm = (np.arange(S)[None, :] < batch_length[bsel, None]).astype(np.float32)
        pem = (pe.T[None, :, :] * lm[:, None, :])       # [2, D, S]
        pem = pem.transpose(1, 0, 2).reshape(D, NT)     # [D, token=(b,s)]
        pem = np.ascontiguousarray(pem.reshape(DT, 128, NT))
        idx = xs.reshape(NT).astype(np.int16)
        idx = np.ascontiguousarray(np.tile(idx.reshape(NT // 16, 16).T, (8, 1)))
        in_maps.append({
            "IDX": idx, "PEM": pem, "KB": kb, "EMB": embed,
            "WQ": wq, "WK": wk, "WV": wv, "WO": wo, "W1": w1, "W2": w2,
            "BO": bo_t, "B1": b1_t, "B2": b2_t,
            "G1": g1_t, "E1": e1_t, "G2": g2_t, "E2": e2_t,
        })
    return in_maps


def kernel(**inputs):
    if "nc" not in _CACHE:
        _CACHE["nc"] = _build()
    nc = _CACHE["nc"]
    in_maps = _host_prep(**inputs)
    res = None
    for attempt in range(3):
        try:
            res = run_bass_kernel_spmd(nc, in_maps, core_ids=list(range(NCORES)))
            break
        except Exception:
            if attempt == 2:
                raise
    out = np.empty((B, S, D), np.float32)
    for c in range(NCORES):
        out[2 * c:2 * c + 2] = res.results[c]["OUT"].reshape(BPC, S, D)
    return out

